# revision 7
# baseline (speedup 1.0000x reference)
"""Trainium2 Bass kernel for the LSTM GAN-discriminator problem.

Math (reference): two 16-step LSTM passes over [B=4096, T=16, F=64] sharing the
first PREV=6 steps (fake sequence = real[:, :6] ++ fake_input), then a dense+
sigmoid head on hidden states of steps 6..15 of each pass.

Strategy:
  - Data parallel: batch 4096 -> 8 cores x 512 rows; weights replicated.
  - Shared prefix: 6 cells at N=512, then the two branches run as separate
    interleaved chains (10 real + 10 fake cells), 26 cell evaluations total.
  - Transposed layout: features on partitions, batch on the free dim. The
    4H=1024 gate columns live as 8 "banks" of 128 partitions; hidden state
    h/c live as 2 slices of 128 partitions concatenated on the free dim, so
    the recurrent matmul contracts K=128 per slice with NO per-step transpose.
  - Gate banks are column-permuted to [i,i,f,f,o,o,g,g] so one big ACT
    sigmoid covers i,f,o and one tanh covers g.
  - Bias is folded into the x-projection via an augmented ones-row (K=65).
  - All matmul operands bf16 (PSUM accumulates fp32); gates/c/h bf16 for the
    2x DVE tensor_tensor mode. Emulated end-to-end rel err ~2.5e-4.
  - Each cell is processed in 2 batch chunks of 256 so PE/ACT/DVE pipeline.
  - Dense head at the end: M=1 matmuls col-packed 4-wide via tile_position
    into PSUM partitions {0,32,64,96}, one sigmoid per group, 4 output DMAs.
"""

import sys

if "/opt/trn_rl_repo" not in sys.path:
    sys.path.insert(0, "/opt/trn_rl_repo")

import numpy as np
import ml_dtypes

import concourse.bass as bass
import concourse.mybir as mybir
import concourse.tile as tile
from concourse import bacc
from concourse.bass_utils import run_bass_kernel_spmd

BF16 = ml_dtypes.bfloat16

PREV, PRED, FEAT, HID = 6, 10, 64, 256
B = 4096
N_CORES = 8
BS = B // N_CORES          # 512 rows per core
CH = 256                   # chunk of the per-core batch
NCH = BS // CH             # 2 chunks
NCELL = PREV + 2 * PRED    # 26 cell evaluations per core
H4 = 4 * HID               # 1024

# gate bank order [i_s0, i_s1, f_s0, f_s1, o_s0, o_s1, g_s0, g_s1]
# (original z column order is i, f, g, o)
_GATE_BASE = [0, 0, 256, 256, 768, 768, 512, 512]
PERM = np.concatenate(
    [np.arange(_GATE_BASE[m] + 128 * (m % 2), _GATE_BASE[m] + 128 * (m % 2) + 128)
     for m in range(8)]
)

# canonical cell ids: 0..5 prefix, 6..15 real steps 6..15, 16..25 fake steps 6..15
# processing order interleaves the two independent branches
CELL_ORDER = list(range(PREV)) + [
    c for t in range(PRED) for c in (PREV + t, PREV + PRED + t)
]


def _h_src(hid_):
    """canonical id of the cell whose h feeds this cell (None for cell 0)."""
    if hid_ == 0:
        return None
    if hid_ == PREV + PRED:  # first fake cell branches off the prefix
        return PREV - 1
    return hid_ - 1


def _build_program():
    f32 = mybir.dt.float32
    bf16 = mybir.dt.bfloat16
    AF = mybir.ActivationFunctionType
    OP = mybir.AluOpType

    nc = bacc.Bacc("TRN2", target_bir_lowering=False, debug=False,
                   num_devices=N_CORES)

    xT = nc.dram_tensor("xT", [NCELL, FEAT + 1, BS], bf16, kind="ExternalInput").ap()
    wx = nc.dram_tensor("wx", [FEAT + 1, H4], bf16, kind="ExternalInput").ap()
    wh = nc.dram_tensor("wh", [128, 2 * H4], bf16, kind="ExternalInput").ap()
    dw = nc.dram_tensor("dw", [128, 2], bf16, kind="ExternalInput").ap()
    dbias = nc.dram_tensor("dbias", [128, 1], mybir.dt.float32,
                           kind="ExternalInput").ap()
    outT = nc.dram_tensor("outT", [2, PRED, BS], f32, kind="ExternalOutput").ap()

    def chunk3(ap_full, ch):
        """[128, 1024] slice-major (s, ch, n) tensor -> [128, 2, 256] chunk view."""
        return ap_full.rearrange("p (s c n) -> p s c n", s=2, c=NCH, n=CH)[:, :, ch]

    def g3(gates, lo):
        """[128, 512] gate range of the chunk-local gates tile -> [128, 2, 256]."""
        return gates[:, lo:lo + 512].rearrange("p (s n) -> p s n", s=2, n=CH)

    with tile.TileContext(nc) as tc:
        with (
            tc.tile_pool(name="const", bufs=1) as const,
            tc.tile_pool(name="xpool", bufs=4) as xpool,
            tc.tile_pool(name="zpool", bufs=2, space="PSUM") as zpool,
            tc.tile_pool(name="gpool", bufs=3) as gpool,
            tc.tile_pool(name="tpool", bufs=3) as tpool,
        ):
            wx_t = const.tile([FEAT + 1, H4], bf16, tag="wx")
            wh_t = const.tile([128, 2 * H4], bf16, tag="wh")
            dw_t = const.tile([128, 2], bf16, tag="dw")
            db_t = const.tile([128, 1], f32, tag="db")
            c_real = const.tile([128, H4], bf16, tag="c_real")
            c_fake = const.tile([128, H4], bf16, tag="c_fake")
            dsig = const.tile([128, 5 * BS], f32, tag="dsig")
            h_tiles = [const.tile([128, H4], bf16, tag=f"h{i}", name=f"h{i}")
                       for i in range(NCELL)]

            nc.sync.dma_start(wx_t[:, :], wx)
            nc.sync.dma_start(wh_t[:, :], wh)
            nc.sync.dma_start(dw_t[:, :], dw)
            nc.sync.dma_start(db_t[:, :], dbias)

            x_tiles = {}
            for hid_ in CELL_ORDER:
                x_t = xpool.tile([FEAT + 1, BS], bf16, tag="x")
                nc.sync.dma_start(x_t[:, :], xT[hid_])
                x_tiles[hid_] = x_t

                h_prev = None if _h_src(hid_) is None else h_tiles[_h_src(hid_)]
                c_buf = c_real if hid_ < PREV + PRED else c_fake

                for ch in range(NCH):
                    z = zpool.tile([128, 8 * CH], f32, tag="z")
                    # --- PE: z = [x;1] @ [Wx;b] + h @ Wh  (8 gate banks) ---
                    for m in range(8):
                        zs = z[:, m * CH:(m + 1) * CH]
                        nc.tensor.matmul(
                            zs,
                            wx_t[:, m * 128:(m + 1) * 128],
                            x_t[:, ch * CH:(ch + 1) * CH],
                            start=True, stop=(h_prev is None),
                        )
                        if h_prev is not None:
                            for s in range(2):
                                nc.tensor.matmul(
                                    zs,
                                    wh_t[:, s * H4 + m * 128: s * H4 + (m + 1) * 128],
                                    h_prev[:, s * 512 + ch * CH: s * 512 + (ch + 1) * CH],
                                    start=False, stop=(s == 1),
                                )
                    # --- ACT: gates ---
                    gates = gpool.tile([128, 8 * CH], bf16, tag="gates")
                    nc.scalar.activation(gates[:, 0:6 * CH], z[:, 0:6 * CH], AF.Sigmoid)
                    nc.scalar.activation(gates[:, 6 * CH:8 * CH], z[:, 6 * CH:8 * CH],
                                         AF.Tanh)
                    # --- DVE: c = f*c + i*g ; h = o*tanh(c) ---
                    i3 = g3(gates, 0)
                    f3 = g3(gates, 512)
                    o3 = g3(gates, 1024)
                    gg3 = g3(gates, 1536)
                    cv = chunk3(c_buf[:, :], ch)
                    if h_prev is None:
                        nc.vector.tensor_tensor(cv, i3, gg3, OP.mult)
                    else:
                        ig = tpool.tile([128, 512], bf16, tag="ig")
                        fc = tpool.tile([128, 512], bf16, tag="fc")
                        ig3 = ig[:, :].rearrange("p (s n) -> p s n", s=2, n=CH)
                        fc3 = fc[:, :].rearrange("p (s n) -> p s n", s=2, n=CH)
                        nc.vector.tensor_tensor(ig3, i3, gg3, OP.mult)
                        nc.vector.tensor_tensor(fc3, f3, cv, OP.mult)
                        nc.vector.tensor_tensor(cv, ig3, fc3, OP.add)
                    tcn = tpool.tile([128, 512], bf16, tag="tc")
                    tc3 = tcn[:, :].rearrange("p (s n) -> p s n", s=2, n=CH)
                    nc.scalar.activation(tc3, cv, AF.Tanh)
                    nc.vector.tensor_tensor(chunk3(h_tiles[hid_][:, :], ch),
                                            o3, tc3, OP.mult)

                if hid_ == PREV - 1:
                    # branch point: fake chain starts from (h5, c5)
                    nc.vector.tensor_copy(c_fake[:, :], c_real[:, :])

            # --- dense head: pred[s] = sigmoid(h_cell @ dw + db) ---
            # slot s = br*10+t -> psum group gi = s%5, partition 32*(s//5)
            for gi in range(5):
                dp = zpool.tile([128, 8 * CH], f32, tag="z")
                for jj in range(4):
                    s_id = jj * 5 + gi
                    br, tp = divmod(s_id, PRED)
                    cell = (PREV if br == 0 else PREV + PRED) + tp
                    for s in range(2):
                        nc.tensor.matmul(
                            dp[32 * jj:32 * jj + 1, 0:BS],
                            dw_t[:, s:s + 1],
                            h_tiles[cell][:, s * 512:(s + 1) * 512],
                            start=(s == 0), stop=(s == 1),
                            tile_position=(0, 32 * jj),
                        )
                nc.scalar.activation(dsig[:, gi * BS:(gi + 1) * BS], dp[:, 0:BS],
                                     AF.Sigmoid, bias=db_t[:, 0:1])

            for jj in range(4):
                br, t0 = jj // 2, 5 * (jj % 2)
                nc.sync.dma_start(outT[br, t0:t0 + 5, :],
                                  dsig[32 * jj:32 * jj + 1, :])

    nc.compile()
    return nc


_PROGRAM = None


def _get_program():
    global _PROGRAM
    if _PROGRAM is None:
        _PROGRAM = _build_program()
    return _PROGRAM


def _prep_inputs(real_input, fake_input, kernel, recurrent_kernel, bias, dense_w,
                 dense_b):
    kernel_p = np.asarray(kernel, np.float32)[:, PERM]
    bias_p = np.asarray(bias, np.float32)[PERM]
    wh_p = np.asarray(recurrent_kernel, np.float32)[:, PERM]

    wx_aug = np.concatenate([kernel_p, bias_p[None]], 0).astype(BF16)  # [65,1024]
    # wh_sb[p, s*1024+j] = wh_p[s*128+p, j]
    wh_sb = np.ascontiguousarray(
        wh_p.reshape(2, 128, H4).transpose(1, 0, 2).reshape(128, 2 * H4)
    ).astype(BF16)
    dw_sb = np.ascontiguousarray(
        np.asarray(dense_w, np.float32)[:, 0].reshape(2, 128).T
    ).astype(BF16)
    db = np.full((128, 1), float(np.asarray(dense_b).reshape(())), np.float32)

    # x cells: 0..15 real steps, 16..25 fake steps; transposed + ones row
    xcat = np.concatenate(
        [np.asarray(real_input, np.float32), np.asarray(fake_input, np.float32)],
        axis=1,
    )  # [B, 26, 64]
    xT = np.transpose(xcat, (1, 2, 0))  # [26, 64, B]
    xT = np.concatenate([xT, np.ones((NCELL, 1, B), np.float32)], axis=1)
    xT = xT.astype(BF16)  # [26, 65, B]

    in_maps = []
    for c in range(N_CORES):
        in_maps.append({
            "xT": np.ascontiguousarray(xT[:, :, c * BS:(c + 1) * BS]),
            "wx": wx_aug,
            "wh": wh_sb,
            "dw": dw_sb,
            "dbias": db,
        })
    return in_maps


def run(inputs, **run_kwargs):
    """Build+run; returns (results_object, (real_pred, fake_pred))."""
    nc = _get_program()
    in_maps = _prep_inputs(**inputs)
    res = run_bass_kernel_spmd(nc, in_maps, list(range(N_CORES)), **run_kwargs)
    outs = res.results
    stacked = np.stack([outs[c]["outT"] for c in range(N_CORES)])  # [8,2,10,512]
    real = stacked[:, 0].transpose(0, 2, 1).reshape(B, PRED, 1)
    fake = stacked[:, 1].transpose(0, 2, 1).reshape(B, PRED, 1)
    return res, (np.asarray(real, np.float32), np.asarray(fake, np.float32))


def kernel(real_input, fake_input, kernel, recurrent_kernel, bias, dense_w,
           dense_b):
    _, out = run(dict(
        real_input=real_input, fake_input=fake_input, kernel=kernel,
        recurrent_kernel=recurrent_kernel, bias=bias, dense_w=dense_w,
        dense_b=dense_b,
    ))
    return out


# revision 9
# speedup vs baseline: 209.4080x; 209.4080x over previous
"""Trainium2 Bass kernel for the LSTM GAN-discriminator problem.

Math (reference): two 16-step LSTM passes over [B=4096, T=16, F=64] sharing the
first PREV=6 steps (fake sequence = real[:, :6] ++ fake_input), then a dense+
sigmoid head on hidden states of steps 6..15 of each pass.

Strategy:
  - Data parallel: batch 4096 -> 8 cores x 512 rows; weights replicated.
  - Shared prefix: 6 cells at N=512, then the two branches run as separate
    interleaved chains (10 real + 10 fake cells), 26 cell evaluations total.
  - Transposed layout: features on partitions, batch on the free dim. The
    4H=1024 gate columns live as 8 "banks" of 128 partitions; hidden state
    h/c live as 2 slices of 128 partitions concatenated on the free dim, so
    the recurrent matmul contracts K=128 per slice with NO per-step transpose.
  - Gate banks are column-permuted to [i,i,f,f,o,o,g,g] so one big ACT
    sigmoid covers i,f,o and one tanh covers g.
  - Bias is folded into the x-projection via an augmented ones-row (K=65).
  - All matmul operands bf16 (PSUM accumulates fp32); gates/c/h bf16 for the
    2x DVE tensor_tensor mode. Emulated end-to-end rel err ~2.5e-4.
  - Each cell is processed in 2 batch chunks of 256 so PE/ACT/DVE pipeline.
  - Dense head at the end: M=1 matmuls col-packed 4-wide via tile_position
    into PSUM partitions {0,32,64,96}, one sigmoid per group, 4 output DMAs.
"""

import sys

if "/opt/trn_rl_repo" not in sys.path:
    sys.path.insert(0, "/opt/trn_rl_repo")

import numpy as np
import ml_dtypes

import concourse.bass as bass
import concourse.mybir as mybir
import concourse.tile as tile
from concourse import bacc
from concourse.bass_utils import run_bass_kernel_spmd

BF16 = ml_dtypes.bfloat16

PREV, PRED, FEAT, HID = 6, 10, 64, 256
B = 4096
N_CORES = 8
BS = B // N_CORES          # 512 rows per core
CH = 256                   # chunk of the per-core batch
NCH = BS // CH             # 2 chunks
NCELL = PREV + 2 * PRED    # 26 cell evaluations per core
H4 = 4 * HID               # 1024

# gate bank order [i_s0, i_s1, f_s0, f_s1, o_s0, o_s1, g_s0, g_s1]
# (original z column order is i, f, g, o)
_GATE_BASE = [0, 0, 256, 256, 768, 768, 512, 512]
PERM = np.concatenate(
    [np.arange(_GATE_BASE[m] + 128 * (m % 2), _GATE_BASE[m] + 128 * (m % 2) + 128)
     for m in range(8)]
)

# canonical cell ids: 0..5 prefix, 6..15 real steps 6..15, 16..25 fake steps 6..15
# processing order interleaves the two independent branches
CELL_ORDER = list(range(PREV)) + [
    c for t in range(PRED) for c in (PREV + t, PREV + PRED + t)
]


def _h_src(hid_):
    """canonical id of the cell whose h feeds this cell (None for cell 0)."""
    if hid_ == 0:
        return None
    if hid_ == PREV + PRED:  # first fake cell branches off the prefix
        return PREV - 1
    return hid_ - 1


def _build_program():
    f32 = mybir.dt.float32
    bf16 = mybir.dt.bfloat16
    AF = mybir.ActivationFunctionType
    OP = mybir.AluOpType

    nc = bacc.Bacc("TRN2", target_bir_lowering=False, debug=False,
                   num_devices=N_CORES)

    xT = nc.dram_tensor("xT", [NCELL, FEAT + 1, BS], bf16, kind="ExternalInput").ap()
    wx = nc.dram_tensor("wx", [FEAT + 1, H4], bf16, kind="ExternalInput").ap()
    wh = nc.dram_tensor("wh", [128, 2 * H4], bf16, kind="ExternalInput").ap()
    dw = nc.dram_tensor("dw", [128, 2], bf16, kind="ExternalInput").ap()
    dbias = nc.dram_tensor("dbias", [128, 1], mybir.dt.float32,
                           kind="ExternalInput").ap()
    outT = nc.dram_tensor("outT", [2, PRED, BS], f32, kind="ExternalOutput").ap()

    def chunk3(ap_full, ch):
        """[128, 1024] slice-major (s, ch, n) tensor -> [128, 2, 256] chunk view."""
        return ap_full.rearrange("p (s c n) -> p s c n", s=2, c=NCH, n=CH)[:, :, ch]

    def g3(gates, lo):
        """[128, 512] gate range of the chunk-local gates tile -> [128, 2, 256]."""
        return gates[:, lo:lo + 512].rearrange("p (s n) -> p s n", s=2, n=CH)

    with tile.TileContext(nc) as tc:
        with (
            tc.tile_pool(name="const", bufs=1) as const,
            tc.tile_pool(name="xpool", bufs=4) as xpool,
            tc.tile_pool(name="zpool", bufs=2, space="PSUM") as zpool,
            tc.tile_pool(name="gpool", bufs=3) as gpool,
            tc.tile_pool(name="tpool", bufs=3) as tpool,
        ):
            wx_t = const.tile([FEAT + 1, H4], bf16, tag="wx")
            wh_t = const.tile([128, 2 * H4], bf16, tag="wh")
            dw_t = const.tile([128, 2], bf16, tag="dw")
            db_t = const.tile([128, 1], f32, tag="db")
            c_real = const.tile([128, H4], bf16, tag="c_real")
            c_fake = const.tile([128, H4], bf16, tag="c_fake")
            dsig = const.tile([128, 5 * BS], f32, tag="dsig")
            h_tiles = [const.tile([128, H4], bf16, tag=f"h{i}", name=f"h{i}")
                       for i in range(NCELL)]

            nc.sync.dma_start(wx_t[:, :], wx)
            nc.sync.dma_start(wh_t[:, :], wh)
            nc.sync.dma_start(dw_t[:, :], dw)
            nc.sync.dma_start(db_t[:, :], dbias)

            x_tiles = {}
            for hid_ in CELL_ORDER:
                x_t = xpool.tile([FEAT + 1, BS], bf16, tag="x")
                nc.sync.dma_start(x_t[:, :], xT[hid_])
                x_tiles[hid_] = x_t

                h_prev = None if _h_src(hid_) is None else h_tiles[_h_src(hid_)]
                c_buf = c_real if hid_ < PREV + PRED else c_fake

                for ch in range(NCH):
                    z = zpool.tile([128, 8 * CH], f32, tag="z")
                    # --- PE: z = [x;1] @ [Wx;b] + h @ Wh  (8 gate banks) ---
                    for m in range(8):
                        zs = z[:, m * CH:(m + 1) * CH]
                        nc.tensor.matmul(
                            zs,
                            wx_t[:, m * 128:(m + 1) * 128],
                            x_t[:, ch * CH:(ch + 1) * CH],
                            start=True, stop=(h_prev is None),
                        )
                        if h_prev is not None:
                            for s in range(2):
                                nc.tensor.matmul(
                                    zs,
                                    wh_t[:, s * H4 + m * 128: s * H4 + (m + 1) * 128],
                                    h_prev[:, s * 512 + ch * CH: s * 512 + (ch + 1) * CH],
                                    start=False, stop=(s == 1),
                                )
                    # --- ACT: gates ---
                    gates = gpool.tile([128, 8 * CH], bf16, tag="gates")
                    nc.scalar.activation(gates[:, 0:6 * CH], z[:, 0:6 * CH], AF.Sigmoid)
                    nc.scalar.activation(gates[:, 6 * CH:8 * CH], z[:, 6 * CH:8 * CH],
                                         AF.Tanh)
                    # --- DVE: c = f*c + i*g ; h = o*tanh(c) ---
                    i3 = g3(gates, 0)
                    f3 = g3(gates, 512)
                    o3 = g3(gates, 1024)
                    gg3 = g3(gates, 1536)
                    cv = chunk3(c_buf[:, :], ch)
                    if h_prev is None:
                        nc.vector.tensor_tensor(cv, i3, gg3, OP.mult)
                    else:
                        ig = tpool.tile([128, 512], bf16, tag="ig")
                        fc = tpool.tile([128, 512], bf16, tag="fc")
                        ig3 = ig[:, :].rearrange("p (s n) -> p s n", s=2, n=CH)
                        fc3 = fc[:, :].rearrange("p (s n) -> p s n", s=2, n=CH)
                        nc.vector.tensor_tensor(ig3, i3, gg3, OP.mult)
                        nc.vector.tensor_tensor(fc3, f3, cv, OP.mult)
                        nc.vector.tensor_tensor(cv, ig3, fc3, OP.add)
                    tcn = tpool.tile([128, 512], bf16, tag="tc")
                    tc3 = tcn[:, :].rearrange("p (s n) -> p s n", s=2, n=CH)
                    nc.scalar.activation(tc3, cv, AF.Tanh)
                    nc.vector.tensor_tensor(chunk3(h_tiles[hid_][:, :], ch),
                                            o3, tc3, OP.mult)

                if hid_ == PREV - 1:
                    # branch point: fake chain starts from (h5, c5)
                    nc.vector.tensor_copy(c_fake[:, :], c_real[:, :])

            # --- dense head: pred[s] = sigmoid(h_cell @ dw + db) ---
            # slot s = br*10+t -> psum group gi = s%5, partition 32*(s//5)
            for gi in range(5):
                dp = zpool.tile([128, 8 * CH], f32, tag="z")
                for jj in range(4):
                    s_id = jj * 5 + gi
                    br, tp = divmod(s_id, PRED)
                    cell = (PREV if br == 0 else PREV + PRED) + tp
                    for s in range(2):
                        nc.tensor.matmul(
                            dp[32 * jj:32 * jj + 1, 0:BS],
                            dw_t[:, s:s + 1],
                            h_tiles[cell][:, s * 512:(s + 1) * 512],
                            start=(s == 0), stop=(s == 1),
                            tile_position=(0, 32 * jj),
                        )
                nc.scalar.activation(dsig[:, gi * BS:(gi + 1) * BS], dp[:, 0:BS],
                                     AF.Sigmoid, bias=db_t[:, 0:1])

            for jj in range(4):
                br, t0 = jj // 2, 5 * (jj % 2)
                nc.sync.dma_start(outT[br, t0:t0 + 5, :],
                                  dsig[32 * jj:32 * jj + 1, :])

    nc.compile()
    return nc


_PROGRAM = None


def _get_program():
    global _PROGRAM
    if _PROGRAM is None:
        _PROGRAM = _build_program()
    return _PROGRAM


def _prep_inputs(real_input, fake_input, kernel, recurrent_kernel, bias, dense_w,
                 dense_b):
    kernel_p = np.asarray(kernel, np.float32)[:, PERM]
    bias_p = np.asarray(bias, np.float32)[PERM]
    wh_p = np.asarray(recurrent_kernel, np.float32)[:, PERM]

    wx_aug = np.concatenate([kernel_p, bias_p[None]], 0).astype(BF16)  # [65,1024]
    # wh_sb[p, s*1024+j] = wh_p[s*128+p, j]
    wh_sb = np.ascontiguousarray(
        wh_p.reshape(2, 128, H4).transpose(1, 0, 2).reshape(128, 2 * H4)
    ).astype(BF16)
    dw_sb = np.ascontiguousarray(
        np.asarray(dense_w, np.float32)[:, 0].reshape(2, 128).T
    ).astype(BF16)
    db = np.full((128, 1), float(np.asarray(dense_b).reshape(())), np.float32)

    # x cells: 0..15 real steps, 16..25 fake steps; transposed + ones row
    xcat = np.concatenate(
        [np.asarray(real_input, np.float32), np.asarray(fake_input, np.float32)],
        axis=1,
    )  # [B, 26, 64]
    xT = np.transpose(xcat, (1, 2, 0))  # [26, 64, B]
    xT = np.concatenate([xT, np.ones((NCELL, 1, B), np.float32)], axis=1)
    xT = xT.astype(BF16)  # [26, 65, B]

    in_maps = []
    for c in range(N_CORES):
        in_maps.append({
            "xT": np.ascontiguousarray(xT[:, :, c * BS:(c + 1) * BS]),
            "wx": wx_aug,
            "wh": wh_sb,
            "dw": dw_sb,
            "dbias": db,
        })
    return in_maps


_EXEC = None


def _get_exec():
    """Cached shard_map executable over the 8 cores (mirrors
    bass2jax.run_bass_via_pjrt but reusable across calls)."""
    global _EXEC
    if _EXEC is not None:
        return _EXEC

    import jax
    from jax.sharding import Mesh, PartitionSpec, NamedSharding
    from jax.experimental.shard_map import shard_map
    from concourse.bass2jax import (_bass_exec_p, install_neuronx_cc_hook,
                                    partition_id_tensor)

    install_neuronx_cc_hook()
    nc = _get_program()

    partition_name = nc.partition_id_tensor.name if nc.partition_id_tensor else None
    in_names, out_names, out_avals, zero_outs = [], [], [], []
    for alloc in nc.m.functions[0].allocations:
        if not isinstance(alloc, mybir.MemoryLocationSet):
            continue
        name = alloc.memorylocations[0].name
        if alloc.kind == "ExternalInput":
            if name != partition_name:
                in_names.append(name)
        elif alloc.kind == "ExternalOutput":
            out_names.append(name)
            shape = tuple(alloc.tensor_shape)
            dtype = mybir.dt.np(alloc.dtype)
            out_avals.append(jax.core.ShapedArray(shape, dtype))
            zero_outs.append(np.zeros(shape, dtype))
    n_params = len(in_names)
    all_in_names = in_names + out_names
    if partition_name is not None:
        all_in_names = all_in_names + [partition_name]

    def _body(*args):
        operands = list(args)
        if partition_name is not None:
            operands.append(partition_id_tensor())
        outs = _bass_exec_p.bind(
            *operands,
            out_avals=tuple(out_avals),
            in_names=tuple(all_in_names),
            out_names=tuple(out_names),
            lowering_input_output_aliases=(),
            sim_require_finite=True,
            sim_require_nnan=True,
            nc=nc,
        )
        return tuple(outs)

    devices = jax.devices()[:N_CORES]
    mesh = Mesh(np.asarray(devices), ("core",))
    n_args = n_params + len(out_names)
    fn = jax.jit(
        shard_map(_body, mesh=mesh,
                  in_specs=(PartitionSpec("core"),) * n_args,
                  out_specs=(PartitionSpec("core"),) * len(out_names),
                  check_rep=False),
        keep_unused=True,
    )
    sharding = NamedSharding(mesh, PartitionSpec("core"))
    _EXEC = dict(fn=fn, in_names=in_names, out_names=out_names,
                 out_avals=out_avals, zero_outs=zero_outs, sharding=sharding)
    return _EXEC


def _concat_args(ex, in_maps):
    args = [
        np.concatenate([np.asarray(m[name]) for m in in_maps], axis=0)
        for name in ex["in_names"]
    ]
    args += [
        np.zeros((N_CORES * z.shape[0], *z.shape[1:]), z.dtype)
        for z in ex["zero_outs"]
    ]
    return args


def _split_out(ex, out_arrs):
    stacked = np.asarray(out_arrs[0]).reshape(N_CORES, 2, PRED, BS)
    real = stacked[:, 0].transpose(0, 2, 1).reshape(B, PRED, 1)
    fake = stacked[:, 1].transpose(0, 2, 1).reshape(B, PRED, 1)
    return np.asarray(real, np.float32), np.asarray(fake, np.float32)


def run(inputs):
    """Run once; returns (real_pred, fake_pred)."""
    ex = _get_exec()
    in_maps = _prep_inputs(**inputs)
    out_arrs = ex["fn"](*_concat_args(ex, in_maps))
    return _split_out(ex, out_arrs)


def bench(inputs, iters=32):
    """Steady-state timing: device-resident args, async dispatch loop."""
    import jax
    import time

    ex = _get_exec()
    in_maps = _prep_inputs(**inputs)
    args = [jax.device_put(a, ex["sharding"]) for a in _concat_args(ex, in_maps)]
    for a in args:
        a.block_until_ready()

    out = ex["fn"](*args)  # warmup / compile
    jax.block_until_ready(out)

    def loop(n):
        t0 = time.perf_counter()
        for _ in range(n):
            out = ex["fn"](*args)
        jax.block_until_ready(out)
        return (time.perf_counter() - t0) / n

    t1 = loop(1)
    tn = loop(iters)
    return tn, t1


def kernel(real_input, fake_input, kernel, recurrent_kernel, bias, dense_w,
           dense_b):
    return run(dict(
        real_input=real_input, fake_input=fake_input, kernel=kernel,
        recurrent_kernel=recurrent_kernel, bias=bias, dense_w=dense_w,
        dense_b=dense_b,
    ))


# revision 22
# speedup vs baseline: 5295.6690x; 25.2888x over previous
"""Trainium2 Bass kernel for the LSTM GAN-discriminator problem.

Math (reference): two 16-step LSTM passes over [B=4096, T=16, F=64] sharing the
first PREV=6 steps (fake sequence = real[:, :6] ++ fake_input), then a dense+
sigmoid head on hidden states of steps 6..15 of each pass.

Strategy:
  - Data parallel: batch 4096 -> 8 cores x 512 rows; weights replicated.
  - Shared prefix: 6 cells at N=512, then the two branches run as separate
    interleaved chains (10 real + 10 fake cells), 26 cell evaluations total.
  - Transposed layout: features on partitions, batch on the free dim. The
    4H=1024 gate columns live as 8 "banks" of 128 partitions; hidden state
    h/c live as 2 slices of 128 partitions concatenated on the free dim, so
    the recurrent matmul contracts K=128 per slice with NO per-step transpose.
  - Gate banks are column-permuted to [i,i,f,f,o,o,g,g] so one big ACT
    sigmoid covers i,f,o and one tanh covers g.
  - Bias is folded into the x-projection via an augmented ones-row (K=65).
  - All matmul operands bf16 (PSUM accumulates fp32); gates/c/h bf16 for the
    2x DVE tensor_tensor mode. Emulated end-to-end rel err ~2.5e-4.
  - Each cell is processed in 2 batch chunks of 256 so PE/ACT/DVE pipeline.
  - Dense head at the end: M=1 matmuls col-packed 4-wide via tile_position
    into PSUM partitions {0,32,64,96}, one sigmoid per group, 4 output DMAs.
"""

import sys

if "/opt/trn_rl_repo" not in sys.path:
    sys.path.insert(0, "/opt/trn_rl_repo")

import numpy as np
import ml_dtypes

import concourse.bass as bass
import concourse.mybir as mybir
import concourse.tile as tile
from concourse import bacc
from concourse.bass_utils import run_bass_kernel_spmd

BF16 = ml_dtypes.bfloat16

PREV, PRED, FEAT, HID = 6, 10, 64, 256
B = 4096
N_CORES = 8
BS = B // N_CORES          # 512 rows per core
CH = 256                   # chunk of the per-core batch
NCH = BS // CH             # 2 chunks
NCELL = PREV + 2 * PRED    # 26 cell evaluations per core
H4 = 4 * HID               # 1024

# gate bank order [i_s0, i_s1, f_s0, f_s1, o_s0, o_s1, g_s0, g_s1]
# (original z column order is i, f, g, o)
_GATE_BASE = [0, 0, 256, 256, 768, 768, 512, 512]
PERM = np.concatenate(
    [np.arange(_GATE_BASE[m] + 128 * (m % 2), _GATE_BASE[m] + 128 * (m % 2) + 128)
     for m in range(8)]
)

# canonical cell ids: 0..5 prefix, 6..15 real steps 6..15, 16..25 fake steps 6..15
# processing order interleaves the two independent branches
CELL_ORDER = list(range(PREV)) + [
    c for t in range(PRED) for c in (PREV + t, PREV + PRED + t)
]


def _h_src(hid_):
    """canonical id of the cell whose h feeds this cell (None for cell 0)."""
    if hid_ == 0:
        return None
    if hid_ == PREV + PRED:  # first fake cell branches off the prefix
        return PREV - 1
    return hid_ - 1


def _build_program(loop_r=None):
    f32 = mybir.dt.float32
    bf16 = mybir.dt.bfloat16
    AF = mybir.ActivationFunctionType
    OP = mybir.AluOpType

    nc = bacc.Bacc("TRN2", target_bir_lowering=False, debug=False,
                   num_devices=N_CORES)

    xT = nc.dram_tensor("xT", [NCELL, FEAT + 1, BS], bf16, kind="ExternalInput").ap()
    wx = nc.dram_tensor("wx", [FEAT + 1, H4], bf16, kind="ExternalInput").ap()
    wh = nc.dram_tensor("wh", [128, 2 * H4], bf16, kind="ExternalInput").ap()
    dw = nc.dram_tensor("dw", [128, 2], bf16, kind="ExternalInput").ap()
    dbias = nc.dram_tensor("dbias", [128, 1], mybir.dt.float32,
                           kind="ExternalInput").ap()
    outT = nc.dram_tensor("outT", [2, PRED, BS], f32, kind="ExternalOutput").ap()

    def chunk3(ap_full, ch):
        """[128, 1024] slice-major (s, ch, n) tensor -> [128, 2, 256] chunk view."""
        return ap_full.rearrange("p (s c n) -> p s c n", s=2, c=NCH, n=CH)[:, :, ch]

    def g3(gates, lo):
        """[128, 512] gate range of the chunk-local gates tile -> [128, 2, 256]."""
        return gates[:, lo:lo + 512].rearrange("p (s n) -> p s n", s=2, n=CH)

    with tile.TileContext(nc) as tc:
        with (
            tc.tile_pool(name="const", bufs=1) as const,
            tc.tile_pool(name="xpool", bufs=4) as xpool,
            tc.tile_pool(name="zpool", bufs=2, space="PSUM") as zpool,
            tc.tile_pool(name="gpool", bufs=3) as gpool,
            tc.tile_pool(name="tpool", bufs=3) as tpool,
        ):
            wx_t = const.tile([FEAT + 1, H4], bf16, tag="wx")
            wh_t = const.tile([128, 2 * H4], bf16, tag="wh")
            dw_t = const.tile([128, 2], bf16, tag="dw")
            db_t = const.tile([128, 1], f32, tag="db")
            c_real = const.tile([128, H4], bf16, tag="c_real")
            c_fake = const.tile([128, H4], bf16, tag="c_fake")
            dsig = const.tile([128, 5 * BS], f32, tag="dsig")
            h_tiles = [const.tile([128, H4], bf16, tag=f"h{i}", name=f"h{i}")
                       for i in range(NCELL)]

            nc.sync.dma_start(wx_t[:, :], wx)
            nc.sync.dma_start(wh_t[:, :], wh)
            nc.sync.dma_start(dw_t[:, :], dw)
            nc.sync.dma_start(db_t[:, :], dbias)

            def emit_body():
              x_tiles = {}
              for hid_ in CELL_ORDER:
                x_t = xpool.tile([FEAT + 1, BS], bf16, tag="x", name="x")
                nc.sync.dma_start(x_t[:, :], xT[hid_])
                x_tiles[hid_] = x_t

                h_prev = None if _h_src(hid_) is None else h_tiles[_h_src(hid_)]
                c_buf = c_real if hid_ < PREV + PRED else c_fake

                for ch in range(NCH):
                    z = zpool.tile([128, 8 * CH], f32, tag="z", name="z")
                    # --- PE: z = [x;1] @ [Wx;b] + h @ Wh  (8 gate banks) ---
                    for m in range(8):
                        zs = z[:, m * CH:(m + 1) * CH]
                        nc.tensor.matmul(
                            zs,
                            wx_t[:, m * 128:(m + 1) * 128],
                            x_t[:, ch * CH:(ch + 1) * CH],
                            start=True, stop=(h_prev is None),
                        )
                        if h_prev is not None:
                            for s in range(2):
                                nc.tensor.matmul(
                                    zs,
                                    wh_t[:, s * H4 + m * 128: s * H4 + (m + 1) * 128],
                                    h_prev[:, s * 512 + ch * CH: s * 512 + (ch + 1) * CH],
                                    start=False, stop=(s == 1),
                                )
                    # --- ACT: gates ---
                    gates = gpool.tile([128, 8 * CH], bf16, tag="gates",
                                       name="gates")
                    nc.scalar.activation(gates[:, 0:6 * CH], z[:, 0:6 * CH], AF.Sigmoid)
                    nc.scalar.activation(gates[:, 6 * CH:8 * CH], z[:, 6 * CH:8 * CH],
                                         AF.Tanh)
                    # --- DVE: c = f*c + i*g ; h = o*tanh(c) ---
                    i3 = g3(gates, 0)
                    f3 = g3(gates, 512)
                    o3 = g3(gates, 1024)
                    gg3 = g3(gates, 1536)
                    cv = chunk3(c_buf[:, :], ch)
                    if h_prev is None:
                        nc.vector.tensor_tensor(cv, i3, gg3, OP.mult)
                    else:
                        ig = tpool.tile([128, 512], bf16, tag="ig", name="ig")
                        fc = tpool.tile([128, 512], bf16, tag="fc", name="fc")
                        ig3 = ig[:, :].rearrange("p (s n) -> p s n", s=2, n=CH)
                        fc3 = fc[:, :].rearrange("p (s n) -> p s n", s=2, n=CH)
                        nc.vector.tensor_tensor(ig3, i3, gg3, OP.mult)
                        nc.vector.tensor_tensor(fc3, f3, cv, OP.mult)
                        nc.vector.tensor_tensor(cv, ig3, fc3, OP.add)
                    tcn = tpool.tile([128, 512], bf16, tag="tc", name="tc")
                    tc3 = tcn[:, :].rearrange("p (s n) -> p s n", s=2, n=CH)
                    nc.scalar.activation(tc3, cv, AF.Tanh)
                    nc.vector.tensor_tensor(chunk3(h_tiles[hid_][:, :], ch),
                                            o3, tc3, OP.mult)

                if hid_ == PREV - 1:
                    # branch point: fake chain starts from (h5, c5)
                    nc.vector.tensor_copy(c_fake[:, :], c_real[:, :])

              # --- dense head: pred[s] = sigmoid(h_cell @ dw + db) ---
              # slot s = br*10+t -> psum group gi = s%5, partition 32*(s//5)
              for gi in range(5):
                dp = zpool.tile([128, 8 * CH], f32, tag="z", name="dp")
                for jj in range(4):
                    s_id = jj * 5 + gi
                    br, tp = divmod(s_id, PRED)
                    cell = (PREV if br == 0 else PREV + PRED) + tp
                    for s in range(2):
                        nc.tensor.matmul(
                            dp[32 * jj:32 * jj + 1, 0:BS],
                            dw_t[:, s:s + 1],
                            h_tiles[cell][:, s * 512:(s + 1) * 512],
                            start=(s == 0), stop=(s == 1),
                            tile_position=(0, 32 * jj),
                        )
                nc.scalar.activation(dsig[:, gi * BS:(gi + 1) * BS], dp[:, 0:BS],
                                     AF.Sigmoid, bias=db_t[:, 0:1])

              for jj in range(4):
                br, t0 = jj // 2, 5 * (jj % 2)
                nc.sync.dma_start(outT[br, t0:t0 + 5, :],
                                  dsig[32 * jj:32 * jj + 1, :])

            if loop_r is None:
                emit_body()
            else:
                with tc.For_i(0, loop_r, 1):
                    emit_body()

    nc.compile()
    return nc


_PROGRAMS = {}


def _get_program(loop_r=None):
    if loop_r not in _PROGRAMS:
        _PROGRAMS[loop_r] = _build_program(loop_r)
    return _PROGRAMS[loop_r]


def _prep_inputs(real_input, fake_input, kernel, recurrent_kernel, bias, dense_w,
                 dense_b):
    kernel_p = np.asarray(kernel, np.float32)[:, PERM]
    bias_p = np.asarray(bias, np.float32)[PERM]
    wh_p = np.asarray(recurrent_kernel, np.float32)[:, PERM]

    wx_aug = np.concatenate([kernel_p, bias_p[None]], 0).astype(BF16)  # [65,1024]
    # wh_sb[p, s*1024+j] = wh_p[s*128+p, j]
    wh_sb = np.ascontiguousarray(
        wh_p.reshape(2, 128, H4).transpose(1, 0, 2).reshape(128, 2 * H4)
    ).astype(BF16)
    dw_sb = np.ascontiguousarray(
        np.asarray(dense_w, np.float32)[:, 0].reshape(2, 128).T
    ).astype(BF16)
    db = np.full((128, 1), float(np.asarray(dense_b).reshape(())), np.float32)

    # x cells: 0..15 real steps, 16..25 fake steps; transposed + ones row
    xcat = np.concatenate(
        [np.asarray(real_input, np.float32), np.asarray(fake_input, np.float32)],
        axis=1,
    )  # [B, 26, 64]
    xT = np.transpose(xcat, (1, 2, 0))  # [26, 64, B]
    xT = np.concatenate([xT, np.ones((NCELL, 1, B), np.float32)], axis=1)
    xT = xT.astype(BF16)  # [26, 65, B]

    in_maps = []
    for c in range(N_CORES):
        in_maps.append({
            "xT": np.ascontiguousarray(xT[:, :, c * BS:(c + 1) * BS]),
            "wx": wx_aug,
            "wh": wh_sb,
            "dw": dw_sb,
            "dbias": db,
        })
    return in_maps


_EXECS = {}


def _get_exec(loop_r=None):
    """Cached shard_map executable over the 8 cores (mirrors
    bass2jax.run_bass_via_pjrt but reusable across calls)."""
    if loop_r in _EXECS:
        return _EXECS[loop_r]

    import jax
    from jax.sharding import Mesh, PartitionSpec, NamedSharding
    from jax.experimental.shard_map import shard_map
    from concourse.bass2jax import (_bass_exec_p, install_neuronx_cc_hook,
                                    partition_id_tensor)

    install_neuronx_cc_hook()
    nc = _get_program(loop_r)

    partition_name = nc.partition_id_tensor.name if nc.partition_id_tensor else None
    in_names, out_names, out_avals, zero_outs = [], [], [], []
    for alloc in nc.m.functions[0].allocations:
        if not isinstance(alloc, mybir.MemoryLocationSet):
            continue
        name = alloc.memorylocations[0].name
        if alloc.kind == "ExternalInput":
            if name != partition_name:
                in_names.append(name)
        elif alloc.kind == "ExternalOutput":
            out_names.append(name)
            shape = tuple(alloc.tensor_shape)
            dtype = mybir.dt.np(alloc.dtype)
            out_avals.append(jax.core.ShapedArray(shape, dtype))
            zero_outs.append(np.zeros(shape, dtype))
    n_params = len(in_names)
    all_in_names = in_names + out_names
    if partition_name is not None:
        all_in_names = all_in_names + [partition_name]

    def _body(*args):
        operands = list(args)
        if partition_name is not None:
            operands.append(partition_id_tensor())
        outs = _bass_exec_p.bind(
            *operands,
            out_avals=tuple(out_avals),
            in_names=tuple(all_in_names),
            out_names=tuple(out_names),
            lowering_input_output_aliases=(),
            sim_require_finite=True,
            sim_require_nnan=True,
            nc=nc,
        )
        return tuple(outs)

    devices = jax.devices()[:N_CORES]
    mesh = Mesh(np.asarray(devices), ("core",))
    n_args = n_params + len(out_names)
    fn = jax.jit(
        shard_map(_body, mesh=mesh,
                  in_specs=(PartitionSpec("core"),) * n_args,
                  out_specs=(PartitionSpec("core"),) * len(out_names),
                  check_rep=False),
        keep_unused=True,
    )
    sharding = NamedSharding(mesh, PartitionSpec("core"))
    _EXECS[loop_r] = dict(fn=fn, in_names=in_names, out_names=out_names,
                          out_avals=out_avals, zero_outs=zero_outs,
                          sharding=sharding)
    return _EXECS[loop_r]


def _concat_args(ex, in_maps):
    args = [
        np.concatenate([np.asarray(m[name]) for m in in_maps], axis=0)
        for name in ex["in_names"]
    ]
    args += [
        np.zeros((N_CORES * z.shape[0], *z.shape[1:]), z.dtype)
        for z in ex["zero_outs"]
    ]
    return args


def _split_out(ex, out_arrs):
    stacked = np.asarray(out_arrs[0]).reshape(N_CORES, 2, PRED, BS)
    real = stacked[:, 0].transpose(0, 2, 1).reshape(B, PRED, 1)
    fake = stacked[:, 1].transpose(0, 2, 1).reshape(B, PRED, 1)
    return np.asarray(real, np.float32), np.asarray(fake, np.float32)


def run(inputs):
    """Run once; returns (real_pred, fake_pred)."""
    ex = _get_exec()
    in_maps = _prep_inputs(**inputs)
    out_arrs = ex["fn"](*_concat_args(ex, in_maps))
    return _split_out(ex, out_arrs)


def bench(inputs, iters=32):
    """Steady-state timing: device-resident args, async dispatch loop."""
    tn, _ = _bench_exec(None, inputs, iters)
    return tn, tn


def _bench_exec(loop_r, inputs, iters):
    import jax
    import time

    ex = _get_exec(loop_r)
    in_maps = _prep_inputs(**inputs)
    args = [jax.device_put(a, ex["sharding"]) for a in _concat_args(ex, in_maps)]
    for a in args:
        a.block_until_ready()

    out = ex["fn"](*args)  # warmup / compile
    jax.block_until_ready(out)

    def loop(n):
        t0 = time.perf_counter()
        for _ in range(n):
            out = ex["fn"](*args)
        jax.block_until_ready(out)
        return (time.perf_counter() - t0) / n

    loop(2)
    best = min(loop(iters) for _ in range(3))
    return best, ex


def bench_hw(inputs, r_hi=64, r_lo=1, iters=8):
    """Per-NEFF-iteration HW time via in-kernel For_i loop: builds two
    program variants (r_hi and r_lo body repeats) and differences the
    per-dispatch times to cancel dispatch/RPC overhead."""
    t_hi, _ = _bench_exec(r_hi, inputs, iters)
    t_lo, _ = _bench_exec(r_lo, inputs, iters)
    return (t_hi - t_lo) / (r_hi - r_lo), t_hi, t_lo


def kernel(real_input, fake_input, kernel, recurrent_kernel, bias, dense_w,
           dense_b):
    return run(dict(
        real_input=real_input, fake_input=fake_input, kernel=kernel,
        recurrent_kernel=recurrent_kernel, bias=bias, dense_w=dense_w,
        dense_b=dense_b,
    ))


# revision 30
# speedup vs baseline: 5498.2457x; 1.0383x over previous
"""Trainium2 Bass kernel for the LSTM GAN-discriminator problem.

Math (reference): two 16-step LSTM passes over [B=4096, T=16, F=64] sharing the
first PREV=6 steps (fake sequence = real[:, :6] ++ fake_input), then a dense+
sigmoid head on hidden states of steps 6..15 of each pass.

Strategy:
  - Data parallel: batch 4096 -> 8 cores x 512 rows; weights replicated.
  - Shared prefix: 6 cells at N=512, then the two branches run as separate
    interleaved chains (10 real + 10 fake cells), 26 cell evaluations total.
  - Transposed layout: features on partitions, batch on the free dim. The
    4H=1024 gate columns live as 8 "banks" of 128 partitions; hidden state
    h/c live as 2 slices of 128 partitions concatenated on the free dim, so
    the recurrent matmul contracts K=128 per slice with NO per-step transpose.
  - Gate banks are column-permuted to [i,i,f,f,o,o,g,g] so one big ACT
    sigmoid covers i,f,o and one tanh covers g.
  - Bias is folded into the x-projection via an augmented ones-row (K=65).
  - All matmul operands bf16 (PSUM accumulates fp32); gates/c/h bf16 for the
    2x DVE tensor_tensor mode. Emulated end-to-end rel err ~2.5e-4.
  - Each cell is processed in 2 batch chunks of 256 so PE/ACT/DVE pipeline.
  - Dense head at the end: M=1 matmuls col-packed 4-wide via tile_position
    into PSUM partitions {0,32,64,96}, one sigmoid per group, 4 output DMAs.
"""

import sys

if "/opt/trn_rl_repo" not in sys.path:
    sys.path.insert(0, "/opt/trn_rl_repo")

import numpy as np
import ml_dtypes

import concourse.bass as bass
import concourse.mybir as mybir
import concourse.tile as tile
from concourse import bacc
from concourse.bass_utils import run_bass_kernel_spmd

BF16 = ml_dtypes.bfloat16

PREV, PRED, FEAT, HID = 6, 10, 64, 256
B = 4096
N_CORES = 8
BS = B // N_CORES          # 512 rows per core
CH = 256                   # chunk of the per-core batch
NCH = BS // CH             # 2 chunks
NCELL = PREV + 2 * PRED    # 26 cell evaluations per core
H4 = 4 * HID               # 1024

# gate bank order [i_s0, i_s1, f_s0, f_s1, o_s0, o_s1, g_s0, g_s1]
# (original z column order is i, f, g, o)
_GATE_BASE = [0, 0, 256, 256, 768, 768, 512, 512]
PERM = np.concatenate(
    [np.arange(_GATE_BASE[m] + 128 * (m % 2), _GATE_BASE[m] + 128 * (m % 2) + 128)
     for m in range(8)]
)

# canonical cell ids: 0..5 prefix, 6..15 real steps 6..15, 16..25 fake steps 6..15
# processing order interleaves the two independent branches
CELL_ORDER = list(range(PREV)) + [
    c for t in range(PRED) for c in (PREV + t, PREV + PRED + t)
]


def _h_src(hid_):
    """canonical id of the cell whose h feeds this cell (None for cell 0)."""
    if hid_ == 0:
        return None
    if hid_ == PREV + PRED:  # first fake cell branches off the prefix
        return PREV - 1
    return hid_ - 1


def _build_program(loop_r=None):
    f32 = mybir.dt.float32
    bf16 = mybir.dt.bfloat16
    AF = mybir.ActivationFunctionType
    OP = mybir.AluOpType

    nc = bacc.Bacc("TRN2", target_bir_lowering=False, debug=False,
                   num_devices=N_CORES)

    xT = nc.dram_tensor("xT", [NCELL, FEAT + 1, BS], bf16, kind="ExternalInput").ap()
    wx = nc.dram_tensor("wx", [FEAT + 1, H4], bf16, kind="ExternalInput").ap()
    wh = nc.dram_tensor("wh", [128, 2 * H4], bf16, kind="ExternalInput").ap()
    dw = nc.dram_tensor("dw", [128, 2], bf16, kind="ExternalInput").ap()
    dbias = nc.dram_tensor("dbias", [128, 1], mybir.dt.float32,
                           kind="ExternalInput").ap()
    outT = nc.dram_tensor("outT", [2, PRED, BS], f32, kind="ExternalOutput").ap()

    def chunk3(ap_full, ch):
        """[128, 1024] slice-major (s, ch, n) tensor -> [128, 2, 256] chunk view."""
        return ap_full.rearrange("p (s c n) -> p s c n", s=2, c=NCH, n=CH)[:, :, ch]

    def g3(gates, lo):
        """[128, 512] gate range of the chunk-local gates tile -> [128, 2, 256]."""
        return gates[:, lo:lo + 512].rearrange("p (s n) -> p s n", s=2, n=CH)

    with tile.TileContext(nc) as tc:
        with (
            tc.tile_pool(name="const", bufs=1) as const,
            tc.tile_pool(name="xpool", bufs=4) as xpool,
            tc.tile_pool(name="zpool", bufs=2, space="PSUM") as zpool,
            tc.tile_pool(name="gpool", bufs=3) as gpool,
            tc.tile_pool(name="tpool", bufs=3) as tpool,
        ):
            wx_t = const.tile([FEAT + 1, H4], bf16, tag="wx")
            wh_t = const.tile([128, 2 * H4], bf16, tag="wh")
            dw_t = const.tile([128, 2], bf16, tag="dw")
            db_t = const.tile([128, 1], f32, tag="db")
            c_real = const.tile([128, H4], bf16, tag="c_real")
            c_fake = const.tile([128, H4], bf16, tag="c_fake")
            dsig = const.tile([128, PRED * BS], f32, tag="dsig")
            h_tiles = [const.tile([128, H4], bf16, tag=f"h{i}", name=f"h{i}")
                       for i in range(NCELL)]

            nc.sync.dma_start(wx_t[:, :], wx)
            nc.sync.dma_start(wh_t[:, :], wh)
            nc.sync.dma_start(dw_t[:, :], dw)
            nc.sync.dma_start(db_t[:, :], dbias)

            def emit_cell(hid_, x_t):
                """One LSTM cell evaluation. Prefix cells run as 2 batch
                chunks of 256 (pipeline depth for the serial chain); branch
                cells run one chunk of 512 (alternating real/fake cells give
                the pipeline parallelism instead). PSUM is split [i,f] /
                [o,g] (4+4 banks at N=512, bufs=1 each) so the next cell's
                matmuls can reuse a tile as soon as its ACT op drains it."""
                h_prev = None if _h_src(hid_) is None else h_tiles[_h_src(hid_)]
                c_buf = c_real if hid_ < PREV + PRED else c_fake
                nch = NCH if hid_ < PREV else 1
                cw = BS // nch

                def mm_bank(ztile, k, m, ch):
                    # accumulate gate bank m (global index) into ztile col k
                    zs = ztile[:, k * cw:(k + 1) * cw]
                    nc.tensor.matmul(
                        zs,
                        wx_t[:, m * 128:(m + 1) * 128],
                        x_t[:, ch * cw:(ch + 1) * cw],
                        start=True, stop=(h_prev is None),
                    )
                    if h_prev is not None:
                        for s in range(2):
                            nc.tensor.matmul(
                                zs,
                                wh_t[:, s * H4 + m * 128: s * H4 + (m + 1) * 128],
                                h_prev[:, s * 512 + ch * cw: s * 512 + (ch + 1) * cw],
                                start=False, stop=(s == 1),
                            )

                for ch in range(nch):
                    zif = zpool.tile([128, 4 * cw], f32, tag="tif", name="zif",
                                     bufs=1)
                    zog = zpool.tile([128, 4 * cw], f32, tag="tog", name="zog",
                                     bufs=1)
                    for m in range(4):
                        mm_bank(zif, m, m, ch)
                    for m in range(4):
                        mm_bank(zog, m, m + 4, ch)
                    # --- ACT: gates (bank order [i i f f] / [o o g g]) ---
                    gif = gpool.tile([128, 4 * cw], bf16, tag="gif", name="gif")
                    gog = gpool.tile([128, 4 * cw], bf16, tag="gog", name="gog")
                    nc.scalar.activation(gif[:, :], zif[:, :], AF.Sigmoid)
                    nc.scalar.activation(gog[:, 2 * cw:4 * cw],
                                         zog[:, 2 * cw:4 * cw], AF.Tanh)
                    nc.scalar.activation(gog[:, 0:2 * cw], zog[:, 0:2 * cw],
                                         AF.Sigmoid)

                    def v3(t, lo):
                        return t[:, lo:lo + 2 * cw].rearrange(
                            "p (s n) -> p s n", s=2, n=cw)

                    i3, f3 = v3(gif, 0), v3(gif, 2 * cw)
                    o3, gg3 = v3(gog, 0), v3(gog, 2 * cw)
                    cv = c_buf[:, :].rearrange("p (s c n) -> p s c n",
                                               s=2, c=nch, n=cw)[:, :, ch]
                    if h_prev is None:
                        nc.vector.tensor_tensor(cv, i3, gg3, OP.mult)
                    else:
                        fc = tpool.tile([128, 2 * cw], bf16, tag="fc", name="fc")
                        ig = tpool.tile([128, 2 * cw], bf16, tag="ig", name="ig")
                        nc.vector.tensor_tensor(v3(fc, 0), f3, cv, OP.mult)
                        nc.vector.tensor_tensor(v3(ig, 0), i3, gg3, OP.mult)
                        nc.vector.tensor_tensor(cv, v3(ig, 0), v3(fc, 0), OP.add)
                    tcn = tpool.tile([128, 2 * cw], bf16, tag="tc", name="tc")
                    nc.scalar.activation(v3(tcn, 0), cv, AF.Tanh)
                    hv = h_tiles[hid_][:, :].rearrange(
                        "p (s c n) -> p s c n", s=2, c=nch, n=cw)[:, :, ch]
                    nc.vector.tensor_tensor(hv, o3, v3(tcn, 0), OP.mult)

            def emit_dense(t):
                """pred[:, t] for both branches: real -> partition 0, fake ->
                partition 32 of one PSUM bank (alternating shared slots so
                consecutive groups pipeline MMs against sigmoids)."""
                dp = zpool.tile([128, BS], f32, tag=("tog" if t % 2 else "tif"),
                                name="dp", bufs=1)
                for br, j0 in ((0, 0), (1, 32)):
                    cell = (PREV if br == 0 else PREV + PRED) + t
                    for s in range(2):
                        nc.tensor.matmul(
                            dp[j0:j0 + 1, 0:BS],
                            dw_t[:, s:s + 1],
                            h_tiles[cell][:, s * 512:(s + 1) * 512],
                            start=(s == 0), stop=(s == 1),
                            tile_position=(0, j0),
                        )
                nc.scalar.activation(dsig[:, t * BS:(t + 1) * BS], dp[:, 0:BS],
                                     AF.Sigmoid, bias=db_t[:, 0:1])

            def emit_body():
              x_tiles = {}
              for hid_ in CELL_ORDER:
                x_t = xpool.tile([FEAT + 1, BS], bf16, tag="x", name="x")
                nc.sync.dma_start(x_t[:, :], xT[hid_])
                x_tiles[hid_] = x_t

                emit_cell(hid_, x_t)

                if hid_ == PREV - 1:
                    # branch point: fake chain starts from (h5, c5)
                    nc.vector.tensor_copy(c_fake[:, :], c_real[:, :])

              for t in range(PRED):
                  emit_dense(t)

              nc.sync.dma_start(outT[0], dsig[0:1, :])
              nc.sync.dma_start(outT[1], dsig[32:33, :])

            if loop_r is None:
                emit_body()
            else:
                with tc.For_i(0, loop_r, 1):
                    emit_body()

    nc.compile()
    return nc


_PROGRAMS = {}


def _get_program(loop_r=None):
    if loop_r not in _PROGRAMS:
        _PROGRAMS[loop_r] = _build_program(loop_r)
    return _PROGRAMS[loop_r]


def _prep_inputs(real_input, fake_input, kernel, recurrent_kernel, bias, dense_w,
                 dense_b):
    kernel_p = np.asarray(kernel, np.float32)[:, PERM]
    bias_p = np.asarray(bias, np.float32)[PERM]
    wh_p = np.asarray(recurrent_kernel, np.float32)[:, PERM]

    wx_aug = np.concatenate([kernel_p, bias_p[None]], 0).astype(BF16)  # [65,1024]
    # wh_sb[p, s*1024+j] = wh_p[s*128+p, j]
    wh_sb = np.ascontiguousarray(
        wh_p.reshape(2, 128, H4).transpose(1, 0, 2).reshape(128, 2 * H4)
    ).astype(BF16)
    dw_sb = np.ascontiguousarray(
        np.asarray(dense_w, np.float32)[:, 0].reshape(2, 128).T
    ).astype(BF16)
    db = np.full((128, 1), float(np.asarray(dense_b).reshape(())), np.float32)

    # x cells: 0..15 real steps, 16..25 fake steps; transposed + ones row
    xcat = np.concatenate(
        [np.asarray(real_input, np.float32), np.asarray(fake_input, np.float32)],
        axis=1,
    )  # [B, 26, 64]
    xT = np.transpose(xcat, (1, 2, 0))  # [26, 64, B]
    xT = np.concatenate([xT, np.ones((NCELL, 1, B), np.float32)], axis=1)
    xT = xT.astype(BF16)  # [26, 65, B]

    in_maps = []
    for c in range(N_CORES):
        in_maps.append({
            "xT": np.ascontiguousarray(xT[:, :, c * BS:(c + 1) * BS]),
            "wx": wx_aug,
            "wh": wh_sb,
            "dw": dw_sb,
            "dbias": db,
        })
    return in_maps


_EXECS = {}


def _get_exec(loop_r=None):
    """Cached shard_map executable over the 8 cores (mirrors
    bass2jax.run_bass_via_pjrt but reusable across calls)."""
    if loop_r in _EXECS:
        return _EXECS[loop_r]

    import jax
    from jax.sharding import Mesh, PartitionSpec, NamedSharding
    from jax.experimental.shard_map import shard_map
    from concourse.bass2jax import (_bass_exec_p, install_neuronx_cc_hook,
                                    partition_id_tensor)

    install_neuronx_cc_hook()
    nc = _get_program(loop_r)

    partition_name = nc.partition_id_tensor.name if nc.partition_id_tensor else None
    in_names, out_names, out_avals, zero_outs = [], [], [], []
    for alloc in nc.m.functions[0].allocations:
        if not isinstance(alloc, mybir.MemoryLocationSet):
            continue
        name = alloc.memorylocations[0].name
        if alloc.kind == "ExternalInput":
            if name != partition_name:
                in_names.append(name)
        elif alloc.kind == "ExternalOutput":
            out_names.append(name)
            shape = tuple(alloc.tensor_shape)
            dtype = mybir.dt.np(alloc.dtype)
            out_avals.append(jax.core.ShapedArray(shape, dtype))
            zero_outs.append(np.zeros(shape, dtype))
    n_params = len(in_names)
    all_in_names = in_names + out_names
    if partition_name is not None:
        all_in_names = all_in_names + [partition_name]

    def _body(*args):
        operands = list(args)
        if partition_name is not None:
            operands.append(partition_id_tensor())
        outs = _bass_exec_p.bind(
            *operands,
            out_avals=tuple(out_avals),
            in_names=tuple(all_in_names),
            out_names=tuple(out_names),
            lowering_input_output_aliases=(),
            sim_require_finite=True,
            sim_require_nnan=True,
            nc=nc,
        )
        return tuple(outs)

    devices = jax.devices()[:N_CORES]
    mesh = Mesh(np.asarray(devices), ("core",))
    n_args = n_params + len(out_names)
    fn = jax.jit(
        shard_map(_body, mesh=mesh,
                  in_specs=(PartitionSpec("core"),) * n_args,
                  out_specs=(PartitionSpec("core"),) * len(out_names),
                  check_rep=False),
        keep_unused=True,
    )
    sharding = NamedSharding(mesh, PartitionSpec("core"))
    _EXECS[loop_r] = dict(fn=fn, in_names=in_names, out_names=out_names,
                          out_avals=out_avals, zero_outs=zero_outs,
                          sharding=sharding)
    return _EXECS[loop_r]


def _concat_args(ex, in_maps):
    args = [
        np.concatenate([np.asarray(m[name]) for m in in_maps], axis=0)
        for name in ex["in_names"]
    ]
    args += [
        np.zeros((N_CORES * z.shape[0], *z.shape[1:]), z.dtype)
        for z in ex["zero_outs"]
    ]
    return args


def _split_out(ex, out_arrs):
    stacked = np.asarray(out_arrs[0]).reshape(N_CORES, 2, PRED, BS)
    real = stacked[:, 0].transpose(0, 2, 1).reshape(B, PRED, 1)
    fake = stacked[:, 1].transpose(0, 2, 1).reshape(B, PRED, 1)
    return np.asarray(real, np.float32), np.asarray(fake, np.float32)


def run(inputs):
    """Run once; returns (real_pred, fake_pred)."""
    ex = _get_exec()
    in_maps = _prep_inputs(**inputs)
    out_arrs = ex["fn"](*_concat_args(ex, in_maps))
    return _split_out(ex, out_arrs)


def bench(inputs, iters=32):
    """Steady-state timing: device-resident args, async dispatch loop."""
    tn, _ = _bench_exec(None, inputs, iters)
    return tn, tn


def _bench_exec(loop_r, inputs, iters):
    import jax
    import time

    ex = _get_exec(loop_r)
    in_maps = _prep_inputs(**inputs)
    args = [jax.device_put(a, ex["sharding"]) for a in _concat_args(ex, in_maps)]
    for a in args:
        a.block_until_ready()

    out = ex["fn"](*args)  # warmup / compile
    jax.block_until_ready(out)

    def loop(n):
        t0 = time.perf_counter()
        for _ in range(n):
            out = ex["fn"](*args)
        jax.block_until_ready(out)
        return (time.perf_counter() - t0) / n

    loop(2)
    best = min(loop(iters) for _ in range(3))
    return best, ex


def bench_hw(inputs, r_hi=64, r_lo=1, iters=8):
    """Per-NEFF-iteration HW time via in-kernel For_i loop: builds two
    program variants (r_hi and r_lo body repeats) and differences the
    per-dispatch times to cancel dispatch/RPC overhead."""
    t_hi, _ = _bench_exec(r_hi, inputs, iters)
    t_lo, _ = _bench_exec(r_lo, inputs, iters)
    return (t_hi - t_lo) / (r_hi - r_lo), t_hi, t_lo


def kernel(real_input, fake_input, kernel, recurrent_kernel, bias, dense_w,
           dense_b):
    return run(dict(
        real_input=real_input, fake_input=fake_input, kernel=kernel,
        recurrent_kernel=recurrent_kernel, bias=bias, dense_w=dense_w,
        dense_b=dense_b,
    ))


# revision 68
# speedup vs baseline: 5759.5212x; 1.0475x over previous
"""Trainium2 Bass kernel for the LSTM GAN-discriminator problem.

Math (reference): two 16-step LSTM passes over [B=4096, T=16, F=64] sharing the
first PREV=6 steps (fake sequence = real[:, :6] ++ fake_input), then a dense+
sigmoid head on hidden states of steps 6..15 of each pass.

Strategy:
  - Data parallel: batch 4096 -> 8 cores x 512 rows; weights replicated.
  - Shared prefix: 6 cells at N=512, then the two branches run as separate
    interleaved chains (10 real + 10 fake cells), 26 cell evaluations total.
  - Transposed layout: features on partitions, batch on the free dim. The
    4H=1024 gate columns live as 8 "banks" of 128 partitions; hidden state
    h/c live as 2 slices of 128 partitions concatenated on the free dim, so
    the recurrent matmul contracts K=128 per slice with NO per-step transpose.
  - Gate banks are column-permuted to [i,i,f,f,o,o,g,g]; PSUM is split into
    an [i,f] tile and an [o,g] tile (4+4 banks at N=512, bufs=1 each) so the
    next cell's matmuls reuse each tile as soon as its ACT ops drain it.
  - Bias is folded into the x-projection via an augmented ones-row (K=65).
  - All matmul operands bf16 (PSUM accumulates fp32); gates/c/h bf16 for the
    2x DVE tensor_tensor mode. Measured end-to-end rel err ~2.3e-4.
    (fp8 DoubleRow for the recurrent matmul was tried: sim-faster but
    HW-slower -- DoubleRow LDWEIGHTS overhead + 1x-rate fp8-out DVE h-op on
    the recurrence chain -- and 7x less accurate. bf16 kept.)
  - Prefix cells run as 2 batch chunks of 256 (pipeline depth for the serial
    chain); branch cells run one chunk of 512, with the two branches
    alternating cells to keep PE/ACT/DVE overlapped.
  - Dense head: step pairs (2t, 2t+1) packed into one 2-bank PSUM tile --
    M=1 matmuls col-packed via tile_position into partitions {0 (real),
    32 (fake)}, pair steps on free halves -- so one sigmoid covers 4 output
    slots; interleaved into the scan ~2 pairs behind the recurrence;
    2 contiguous output DMAs. (Do not use col-groups (0,64)/(0,96):
    quadrant-3 tile_position wedged the device in testing.)
"""

import sys

if "/opt/trn_rl_repo" not in sys.path:
    sys.path.insert(0, "/opt/trn_rl_repo")

import numpy as np
import ml_dtypes

import concourse.mybir as mybir
import concourse.tile as tile
from concourse import bacc

BF16 = ml_dtypes.bfloat16

PREV, PRED, FEAT, HID = 6, 10, 64, 256
B = 4096
N_CORES = 8
BS = B // N_CORES          # 512 rows per core
CH = 256                   # chunk of the per-core batch
NCH = BS // CH             # 2 chunks
NCELL = PREV + 2 * PRED    # 26 cell evaluations per core
H4 = 4 * HID               # 1024

# gate bank order [i_s0, i_s1, f_s0, f_s1, o_s0, o_s1, g_s0, g_s1]
# (original z column order is i, f, g, o)
_GATE_BASE = [0, 0, 256, 256, 768, 768, 512, 512]
PERM = np.concatenate(
    [np.arange(_GATE_BASE[m] + 128 * (m % 2), _GATE_BASE[m] + 128 * (m % 2) + 128)
     for m in range(8)]
)

# canonical cell ids: 0..5 prefix, 6..15 real steps 6..15, 16..25 fake steps 6..15
# processing order interleaves the two independent branches; the fake cell
# goes first in each pair so f6 can read c5 out of c_real before r6
# overwrites it (no c copy needed at the branch point)
CELL_ORDER = list(range(PREV)) + [
    c for t in range(PRED) for c in (PREV + PRED + t, PREV + t)
]


def _h_src(hid_):
    """canonical id of the cell whose h feeds this cell (None for cell 0)."""
    if hid_ == 0:
        return None
    if hid_ == PREV + PRED:  # first fake cell branches off the prefix
        return PREV - 1
    return hid_ - 1


def _build_program(loop_r=None):
    f32 = mybir.dt.float32
    bf16 = mybir.dt.bfloat16
    AF = mybir.ActivationFunctionType
    OP = mybir.AluOpType

    nc = bacc.Bacc("TRN2", target_bir_lowering=False, debug=False,
                   num_devices=N_CORES)

    xT = nc.dram_tensor("xT", [NCELL, FEAT + 1, BS], bf16, kind="ExternalInput").ap()
    wx = nc.dram_tensor("wx", [FEAT + 1, H4], bf16, kind="ExternalInput").ap()
    wh = nc.dram_tensor("wh", [128, 2 * H4], bf16, kind="ExternalInput").ap()
    dw = nc.dram_tensor("dw", [128, 2], bf16, kind="ExternalInput").ap()
    dbias = nc.dram_tensor("dbias", [128, 1], mybir.dt.float32,
                           kind="ExternalInput").ap()
    outT = nc.dram_tensor("outT", [2, PRED, BS], f32, kind="ExternalOutput").ap()

    def chunk3(ap_full, ch):
        """[128, 1024] slice-major (s, ch, n) tensor -> [128, 2, 256] chunk view."""
        return ap_full.rearrange("p (s c n) -> p s c n", s=2, c=NCH, n=CH)[:, :, ch]

    def g3(gates, lo):
        """[128, 512] gate range of the chunk-local gates tile -> [128, 2, 256]."""
        return gates[:, lo:lo + 512].rearrange("p (s n) -> p s n", s=2, n=CH)

    with tile.TileContext(nc) as tc:
        with (
            tc.tile_pool(name="const", bufs=1) as const,
            tc.tile_pool(name="xpool", bufs=4) as xpool,
            tc.tile_pool(name="zpool", bufs=2, space="PSUM") as zpool,
            tc.tile_pool(name="gpool", bufs=3) as gpool,
            tc.tile_pool(name="tpool", bufs=3) as tpool,
        ):
            wx_t = const.tile([FEAT + 1, H4], bf16, tag="wx")
            wh_t = const.tile([128, 2 * H4], bf16, tag="wh")
            dw_t = const.tile([128, 2], bf16, tag="dw")
            db_t = const.tile([128, 1], f32, tag="db")
            c_real = const.tile([128, H4], bf16, tag="c_real")
            c_fake = const.tile([128, H4], bf16, tag="c_fake")
            dsig = const.tile([128, PRED * BS], f32, tag="dsig")
            h_tiles = [const.tile([128, H4], bf16, tag=f"h{i}", name=f"h{i}")
                       for i in range(NCELL)]

            # dummy activation: forces the sigmoid/tanh ACT table load to
            # happen during the weight DMAs instead of on the critical path
            warm = tpool.tile([128, 1], f32, tag="warm", name="warm")
            nc.scalar.activation(warm[:, :], db_t[:, :], AF.Sigmoid)
            nc.scalar.activation(warm[:, :], db_t[:, :], AF.Tanh)

            nc.sync.dma_start(wx_t[:, :], wx)
            nc.sync.dma_start(wh_t[:, :], wh)

            def emit_cell(hid_, x_t):
                """One LSTM cell evaluation. Prefix cells run as 2 batch
                chunks of 256 (pipeline depth for the serial chain); branch
                cells run one chunk of 512 (alternating real/fake cells give
                the pipeline parallelism instead). PSUM is split [i,f] /
                [o,g] (4+4 banks at N=512, bufs=1 each) so the next cell's
                matmuls can reuse a tile as soon as its ACT op drains it."""
                h_prev = None if _h_src(hid_) is None else h_tiles[_h_src(hid_)]
                c_in = c_out = c_real if hid_ < PREV + PRED else c_fake
                if hid_ == PREV + PRED:
                    c_in = c_real  # branch point: fake chain starts from c5
                nch = NCH if hid_ < PREV else 1
                cw = BS // nch

                def mm_bank(ztile, k, m, ch):
                    # accumulate gate bank m (global index) into ztile col k
                    zs = ztile[:, k * cw:(k + 1) * cw]
                    nc.tensor.matmul(
                        zs,
                        wx_t[:, m * 128:(m + 1) * 128],
                        x_t[:, ch * cw:(ch + 1) * cw],
                        start=True, stop=(h_prev is None),
                    )
                    if h_prev is not None:
                        for s in range(2):
                            nc.tensor.matmul(
                                zs,
                                wh_t[:, s * H4 + m * 128: s * H4 + (m + 1) * 128],
                                h_prev[:, s * 512 + ch * cw: s * 512 + (ch + 1) * cw],
                                start=False, stop=(s == 1),
                            )

                for ch in range(nch):
                    zif = zpool.tile([128, 4 * cw], f32, tag="tif", name="zif",
                                     bufs=1)
                    zog = zpool.tile([128, 4 * cw], f32, tag="tog", name="zog",
                                     bufs=1)
                    for m in range(4):
                        mm_bank(zif, m, m, ch)
                    for m in range(4):
                        mm_bank(zog, m, m + 4, ch)
                    # --- ACT: gates (bank order [i i f f] / [o o g g]) ---
                    gif = gpool.tile([128, 4 * cw], bf16, tag="gif", name="gif")
                    gog = gpool.tile([128, 4 * cw], bf16, tag="gog", name="gog")
                    nc.scalar.activation(gif[:, :], zif[:, :], AF.Sigmoid)
                    nc.scalar.activation(gog[:, 2 * cw:4 * cw],
                                         zog[:, 2 * cw:4 * cw], AF.Tanh)
                    nc.scalar.activation(gog[:, 0:2 * cw], zog[:, 0:2 * cw],
                                         AF.Sigmoid)

                    def v3(t, lo):
                        return t[:, lo:lo + 2 * cw].rearrange(
                            "p (s n) -> p s n", s=2, n=cw)

                    i3, f3 = v3(gif, 0), v3(gif, 2 * cw)
                    o3, gg3 = v3(gog, 0), v3(gog, 2 * cw)

                    def cvw(t):
                        return t[:, :].rearrange("p (s c n) -> p s c n",
                                                 s=2, c=nch, n=cw)[:, :, ch]

                    cvi, cvo = cvw(c_in), cvw(c_out)
                    if h_prev is None:
                        nc.vector.tensor_tensor(cvo, i3, gg3, OP.mult)
                    else:
                        fc = tpool.tile([128, 2 * cw], bf16, tag="fc", name="fc")
                        ig = tpool.tile([128, 2 * cw], bf16, tag="ig", name="ig")
                        nc.vector.tensor_tensor(v3(fc, 0), f3, cvi, OP.mult)
                        nc.vector.tensor_tensor(v3(ig, 0), i3, gg3, OP.mult)
                        nc.vector.tensor_tensor(cvo, v3(ig, 0), v3(fc, 0), OP.add)
                    tcn = tpool.tile([128, 2 * cw], bf16, tag="tc", name="tc")
                    nc.scalar.activation(v3(tcn, 0), cvo, AF.Tanh)
                    hv = h_tiles[hid_][:, :].rearrange(
                        "p (s c n) -> p s c n", s=2, c=nch, n=cw)[:, :, ch]
                    nc.vector.tensor_tensor(hv, o3, v3(tcn, 0), OP.mult)

            def emit_dense(tp_):
                """pred[:, 2tp:2tp+2] for both branches: partitions {0 real,
                32 fake} of a 2-bank PSUM tile, the two steps of the pair on
                free halves; ONE sigmoid covers all 4 output slots."""
                dp = zpool.tile([128, 2 * BS], f32,
                                tag=("tog" if tp_ % 2 else "tif"),
                                name="dp", bufs=1)
                for dt_ in (0, 1):
                    for br, j0 in ((0, 0), (1, 32)):
                        cell = (PREV if br == 0 else PREV + PRED) + 2 * tp_ + dt_
                        for s in range(2):
                            nc.tensor.matmul(
                                dp[j0:j0 + 1, dt_ * BS:(dt_ + 1) * BS],
                                dw_t[:, s:s + 1],
                                h_tiles[cell][:, s * 512:(s + 1) * 512],
                                start=(s == 0), stop=(s == 1),
                                tile_position=(0, j0),
                            )
                nc.scalar.activation(dsig[:, 2 * tp_ * BS:(2 * tp_ + 2) * BS],
                                     dp[:, 0:2 * BS],
                                     AF.Sigmoid, bias=db_t[:, 0:1])

            def emit_body():
              x_tiles = {}
              for hid_ in CELL_ORDER:
                x_t = xpool.tile([FEAT + 1, BS], bf16, tag="x", name="x")
                nc.sync.dma_start(x_t[:, :], xT[hid_])
                x_tiles[hid_] = x_t

                emit_cell(hid_, x_t)

                if hid_ == 0:
                    # small weight DMAs off the startup critical path
                    nc.sync.dma_start(dw_t[:, :], dw)
                    nc.sync.dma_start(db_t[:, :], dbias)

                t_r = hid_ - PREV  # real cell completes step t_r
                if PREV <= hid_ < PREV + PRED and t_r >= 5 and t_r % 2 == 1:
                    # dense pair (2tp, 2tp+1), ~2 pairs behind the scan
                    emit_dense((t_r - 5) // 2)

              for tp_ in range(PRED // 2 - 2, PRED // 2):
                  emit_dense(tp_)

              nc.sync.dma_start(outT[0], dsig[0:1, :])
              nc.sync.dma_start(outT[1], dsig[32:33, :])

            if loop_r is None:
                emit_body()
            else:
                with tc.For_i(0, loop_r, 1,
                              hint_engines=(mybir.EngineType.PE,)):
                    emit_body()

    nc.compile()
    return nc


_PROGRAMS = {}


def _get_program(loop_r=None):
    if loop_r not in _PROGRAMS:
        _PROGRAMS[loop_r] = _build_program(loop_r)
    return _PROGRAMS[loop_r]


def _prep_inputs(real_input, fake_input, kernel, recurrent_kernel, bias, dense_w,
                 dense_b):
    kernel_p = np.asarray(kernel, np.float32)[:, PERM]
    bias_p = np.asarray(bias, np.float32)[PERM]
    wh_p = np.asarray(recurrent_kernel, np.float32)[:, PERM]

    wx_aug = np.concatenate([kernel_p, bias_p[None]], 0).astype(BF16)  # [65,1024]
    # wh_sb[p, s*1024+j] = wh_p[s*128+p, j]
    wh_sb = np.ascontiguousarray(
        wh_p.reshape(2, 128, H4).transpose(1, 0, 2).reshape(128, 2 * H4)
    ).astype(BF16)
    dw_sb = np.ascontiguousarray(
        np.asarray(dense_w, np.float32)[:, 0].reshape(2, 128).T
    ).astype(BF16)
    db = np.full((128, 1), float(np.asarray(dense_b).reshape(())), np.float32)

    # x cells: 0..15 real steps, 16..25 fake steps; transposed + ones row
    xcat = np.concatenate(
        [np.asarray(real_input, np.float32), np.asarray(fake_input, np.float32)],
        axis=1,
    )  # [B, 26, 64]
    xT = np.transpose(xcat, (1, 2, 0))  # [26, 64, B]
    xT = np.concatenate([xT, np.ones((NCELL, 1, B), np.float32)], axis=1)
    xT = xT.astype(BF16)  # [26, 65, B]

    in_maps = []
    for c in range(N_CORES):
        in_maps.append({
            "xT": np.ascontiguousarray(xT[:, :, c * BS:(c + 1) * BS]),
            "wx": wx_aug,
            "wh": wh_sb,
            "dw": dw_sb,
            "dbias": db,
        })
    return in_maps


_EXECS = {}


def _get_exec(loop_r=None):
    """Cached shard_map executable over the 8 cores (mirrors
    bass2jax.run_bass_via_pjrt but reusable across calls)."""
    if loop_r in _EXECS:
        return _EXECS[loop_r]

    import jax
    from jax.sharding import Mesh, PartitionSpec, NamedSharding
    from jax.experimental.shard_map import shard_map
    from concourse.bass2jax import (_bass_exec_p, install_neuronx_cc_hook,
                                    partition_id_tensor)

    install_neuronx_cc_hook()
    nc = _get_program(loop_r)

    partition_name = nc.partition_id_tensor.name if nc.partition_id_tensor else None
    in_names, out_names, out_avals, zero_outs = [], [], [], []
    for alloc in nc.m.functions[0].allocations:
        if not isinstance(alloc, mybir.MemoryLocationSet):
            continue
        name = alloc.memorylocations[0].name
        if alloc.kind == "ExternalInput":
            if name != partition_name:
                in_names.append(name)
        elif alloc.kind == "ExternalOutput":
            out_names.append(name)
            shape = tuple(alloc.tensor_shape)
            dtype = mybir.dt.np(alloc.dtype)
            out_avals.append(jax.core.ShapedArray(shape, dtype))
            zero_outs.append(np.zeros(shape, dtype))
    n_params = len(in_names)
    all_in_names = in_names + out_names
    if partition_name is not None:
        all_in_names = all_in_names + [partition_name]

    def _body(*args):
        operands = list(args)
        if partition_name is not None:
            operands.append(partition_id_tensor())
        outs = _bass_exec_p.bind(
            *operands,
            out_avals=tuple(out_avals),
            in_names=tuple(all_in_names),
            out_names=tuple(out_names),
            lowering_input_output_aliases=(),
            sim_require_finite=True,
            sim_require_nnan=True,
            nc=nc,
        )
        return tuple(outs)

    devices = jax.devices()[:N_CORES]
    mesh = Mesh(np.asarray(devices), ("core",))
    n_args = n_params + len(out_names)
    fn = jax.jit(
        shard_map(_body, mesh=mesh,
                  in_specs=(PartitionSpec("core"),) * n_args,
                  out_specs=(PartitionSpec("core"),) * len(out_names),
                  check_rep=False),
        keep_unused=True,
    )
    sharding = NamedSharding(mesh, PartitionSpec("core"))
    _EXECS[loop_r] = dict(fn=fn, in_names=in_names, out_names=out_names,
                          out_avals=out_avals, zero_outs=zero_outs,
                          sharding=sharding)
    return _EXECS[loop_r]


def _concat_args(ex, in_maps):
    args = [
        np.concatenate([np.asarray(m[name]) for m in in_maps], axis=0)
        for name in ex["in_names"]
    ]
    args += [
        np.zeros((N_CORES * z.shape[0], *z.shape[1:]), z.dtype)
        for z in ex["zero_outs"]
    ]
    return args


def _split_out(ex, out_arrs):
    stacked = np.asarray(out_arrs[0]).reshape(N_CORES, 2, PRED, BS)
    real = stacked[:, 0].transpose(0, 2, 1).reshape(B, PRED, 1)
    fake = stacked[:, 1].transpose(0, 2, 1).reshape(B, PRED, 1)
    return np.asarray(real, np.float32), np.asarray(fake, np.float32)


def run(inputs):
    """Run once; returns (real_pred, fake_pred)."""
    ex = _get_exec()
    in_maps = _prep_inputs(**inputs)
    out_arrs = ex["fn"](*_concat_args(ex, in_maps))
    return _split_out(ex, out_arrs)


def bench(inputs, iters=32):
    """Steady-state timing: device-resident args, async dispatch loop."""
    tn, _ = _bench_exec(None, inputs, iters)
    return tn, tn


def _bench_prep(loop_r, inputs):
    import jax

    ex = _get_exec(loop_r)
    in_maps = _prep_inputs(**inputs)
    args = [jax.device_put(a, ex["sharding"]) for a in _concat_args(ex, in_maps)]
    for a in args:
        a.block_until_ready()
    out = ex["fn"](*args)  # warmup / compile
    jax.block_until_ready(out)
    return ex, args


def bench_hw(inputs, r_hi=128, r_lo=8, samples=10):
    """Per-NEFF-iteration HW time via in-kernel For_i loop: min-of-N
    dispatch times for the r_hi and r_lo program variants (measured in
    blocks -- alternating executables forces NEFF reloads), then diff to
    cancel dispatch/RPC overhead."""
    import jax
    import time

    def one(ex, args):
        t0 = time.perf_counter()
        out = ex["fn"](*args)
        jax.block_until_ready(out)
        return time.perf_counter() - t0

    def block(loop_r):
        ex, args = _bench_prep(loop_r, inputs)
        one(ex, args)  # absorb NEFF switch
        return min(one(ex, args) for _ in range(samples))

    t_hi = block(r_hi)
    t_lo = block(r_lo)
    return (t_hi - t_lo) / (r_hi - r_lo), t_hi, t_lo


def kernel(real_input, fake_input, kernel, recurrent_kernel, bias, dense_w,
           dense_b):
    return run(dict(
        real_input=real_input, fake_input=fake_input, kernel=kernel,
        recurrent_kernel=recurrent_kernel, bias=bias, dense_w=dense_w,
        dense_b=dense_b,
    ))
